# revision 1
# baseline (speedup 1.0000x reference)
"""TRN2 Bass kernel for the ConceptualMambaBlock problem.

Math (reference):
    x: [B=4, T=96, N=512, H=128] f32
    expanded = x @ W_exp.T + b_exp            # [B,T,N,2H]
    primary, gating = split(expanded, 2, -1)
    s_t = 0.9*s_{t-1} + 0.1*gating_t          # EMA along T
    out = (primary * sigmoid(s)) @ W_con.T + b_con

Strategy:
  - Shard (B x N/2) over 8 cores: core c -> batch c//2, node half c%2.
  - Host pre-transposes each core's x shard to [H, N_local, T] so the
    contraction dim H lands on SBUF partitions with fully-contiguous DMA;
    no on-chip transposes anywhere.
  - Per 4-node block (tok = 4*96 = 384 columns, t fastest):
      mm1 (fp32r, full PE rate) -> PSUM gating/primary [o=128, tok]
      EMA via DVE tensor_tensor_scan: state = mask*state + g  (mask has 0.0
      at each t=0 column, so the 4 node-segments reset exactly).
      The gating bias is handled without extra matmuls via a sigma-shift:
      scanning sigma_t = s_t - b_g needs only -0.9*b_g added to the t=0
      columns of the gating PSUM (a tiny [128,4] ACT fixup) and +b_g folded
      into the sigmoid's per-partition bias port.
      sigmoid on ACT; gate-mul + primary bias in one DVE op;
      mm2 (fp32r); output bias via ACT Identity; DMA out.
  - Software pipeline over MM-groups of 2 blocks with one iteration of
    slack on every cross-engine edge:
      PE (iter g) : w1g(g) x2 -> w1p(g) x2 -> w2(g-1) x2
      DVE (iter g): stt(g-1) x2 -> scan(g) x2
      ACT (iter g): fixup(g) x2 -> sig(g) x2 -> id(g-1) x2
  - DMA grouped: one load / one store covers 4 consecutive blocks; all
    constants packed into two DMAs.
  - Output returned as [H, N_local, T] per core; host transposes back.
"""

import numpy as np

import concourse.bacc as bacc
import concourse.bass as bass  # noqa: F401  (engine types referenced via nc)
import concourse.mybir as mybir
import concourse.tile as tile
from concourse.bass_utils import run_bass_kernel_spmd

F32 = mybir.dt.float32
F32R = mybir.dt.float32r
AF = mybir.ActivationFunctionType
ALU = mybir.AluOpType

B, T, N, H = 4, 96, 512, 128
NCORES = 8
NLOC = N // 2          # 256 nodes per core
NB = 4                 # nodes per block
TOK = NB * T           # 384 columns per block
NBLK = NLOC // NB      # 64 blocks per core
GRP = 4                # blocks per DMA group
NGRP = NBLK // GRP

_NC_CACHE = None


def _build():
    nc = bacc.Bacc()

    xt_h = nc.dram_tensor("xt", [H, NBLK, TOK], F32R, kind="ExternalInput")
    wpack_h = nc.dram_tensor("wpack", [H, 3 * H], F32R, kind="ExternalInput")
    bpack_h = nc.dram_tensor("bpack", [H, 4], F32, kind="ExternalInput")
    out_h = nc.dram_tensor("out", [H, NBLK, TOK], F32, kind="ExternalOutput")

    with tile.TileContext(nc) as tc:
        with (
            tc.tile_pool(name="consts", bufs=1) as cp,
            tc.tile_pool(name="io", bufs=3) as io,
            tc.tile_pool(name="mid", bufs=8) as mid,
            tc.tile_pool(name="ps", bufs=2, space="PSUM") as ps,
        ):
            wpack_sb = cp.tile([H, 3 * H], F32R, tag="wpack")
            nc.sync.dma_start(out=wpack_sb[:], in_=wpack_h[:, :])
            bpack_sb = cp.tile([H, 4], F32, tag="bpack")
            nc.sync.dma_start(out=bpack_sb[:], in_=bpack_h[:, :])
            w1p_sb = wpack_sb[:, 0:H]
            w1g_sb = wpack_sb[:, H : 2 * H]
            w2_sb = wpack_sb[:, 2 * H : 3 * H]
            bneg_sb = bpack_sb[:, 0:1]
            bg_sb = bpack_sb[:, 1:2]
            b1p_sb = bpack_sb[:, 2:3]
            b2_sb = bpack_sb[:, 3:4]

            mask_sb = cp.tile([H, NB, T], F32, tag="mask")
            nc.gpsimd.memset(mask_sb[:], 0.9)
            nc.gpsimd.memset(mask_sb[:, :, 0:1], 0.0)
            mask2d = mask_sb[:].rearrange("p a b -> p (a b)")

            MG = 2                    # blocks per matmul phase group
            NMG = NBLK // MG          # 32 iterations
            DG = GRP // MG            # MM-groups per DMA group

            state = {}                # per-iteration tiles carried forward

            def emit_stt(g):
                # gate-mul of iteration g (y = (pp + b1p) * sig)
                pps, sgs = state[g]["pps"], state[g]["sgs"]
                ys = []
                for j in range(MG):
                    y = mid.tile([H, TOK], F32R, tag="y", name=f"y{j}")
                    nc.vector.scalar_tensor_tensor(
                        out=y[:], in0=pps[j][:], scalar=b1p_sb, in1=sgs[j][:],
                        op0=ALU.add, op1=ALU.mult,
                    )
                    ys.append(y)
                state[g]["ys"] = ys

            def emit_mm2_and_out(g):
                ys, ob4 = state[g]["ys"], state[g]["ob4"]
                pos = []
                for j in range(MG):
                    po = ps.tile([H, TOK], F32, tag="po", name=f"po{j}", bufs=2)
                    nc.tensor.matmul(
                        po[:], lhsT=w2_sb, rhs=ys[j][:], start=True, stop=True
                    )
                    pos.append(po)
                for j in range(MG):
                    nc.scalar.activation(
                        ob4[:, (g % DG) * MG + j, :], pos[j][:],
                        AF.Identity, bias=b2_sb, scale=1.0,
                    )
                if g % DG == DG - 1:
                    dgi = g // DG
                    nc.gpsimd.dma_start(
                        out=out_h[:, dgi * GRP : (dgi + 1) * GRP, :], in_=ob4[:]
                    )
                del state[g]

            xt4 = None
            ob4 = None
            for g in range(NMG):
                if g % DG == 0:
                    dgi = g // DG
                    xt4 = io.tile([H, GRP, TOK], F32R, tag="xt", name="xt4")
                    if g == 0:
                        half = GRP // 2
                        nc.sync.dma_start(
                            out=xt4[:, :half, :], in_=xt_h[:, :half, :]
                        )
                        nc.sync.dma_start(
                            out=xt4[:, half:, :], in_=xt_h[:, half:GRP, :]
                        )
                    else:
                        nc.sync.dma_start(
                            out=xt4[:], in_=xt_h[:, dgi * GRP : (dgi + 1) * GRP, :]
                        )
                    ob4 = io.tile([H, GRP, TOK], F32, tag="ob", name="ob4")
                xts = [xt4[:, (g % DG) * MG + j, :] for j in range(MG)]
                state[g] = {"ob4": ob4}

                # PE: gating phase
                pgs = [ps.tile([H, TOK], F32, tag="pg", name=f"pg{j}", bufs=3)
                       for j in range(MG)]
                for j in range(MG):
                    nc.tensor.matmul(
                        pgs[j][:], lhsT=w1g_sb, rhs=xts[j], start=True, stop=True
                    )
                # sigma-shift fixup: t=0 columns of the gating PSUM get -0.9*b_g
                # (the remaining +b_g shift is folded into the sigmoid bias)
                for j in range(MG):
                    pgc = pgs[j][:].rearrange("p (a b) -> p a b", b=T)[:, :, 0:1]
                    nc.scalar.activation(pgc, pgc, AF.Identity, bias=bneg_sb, scale=1.0)

                # DVE: previous iteration's gate-mul first (deps long ready)
                if g - 1 in state and "sgs" in state.get(g - 1, {}):
                    emit_stt(g - 1)

                # PE: primary phase
                pps = [ps.tile([H, TOK], F32, tag="pp", name=f"pp{j}", bufs=3)
                       for j in range(MG)]
                for j in range(MG):
                    nc.tensor.matmul(
                        pps[j][:], lhsT=w1p_sb, rhs=xts[j], start=True, stop=True
                    )
                state[g]["pps"] = pps

                # DVE: this iteration's scans
                ss = []
                for j in range(MG):
                    s = mid.tile([H, TOK], F32, tag="s", name=f"s{j}")
                    nc.vector.tensor_tensor_scan(
                        out=s[:], data0=mask2d, data1=pgs[j][:],
                        initial=0.0, op0=ALU.mult, op1=ALU.add,
                    )
                    ss.append(s)
                # ACT: sigmoids
                sgs = []
                for j in range(MG):
                    sg = mid.tile([H, TOK], F32, tag="sg", name=f"sg{j}")
                    nc.scalar.activation(sg[:], ss[j][:], AF.Sigmoid, bias=bg_sb, scale=1.0)
                    sgs.append(sg)
                state[g]["sgs"] = sgs

                # PE: mm2 of g-1 (y produced by the stt emitted above)
                if g - 1 in state and "ys" in state.get(g - 1, {}):
                    emit_mm2_and_out(g - 1)

            # drain: stt + mm2 of the last iteration
            emit_stt(NMG - 1)
            emit_mm2_and_out(NMG - 1)

    nc.finalize()
    return nc


def _get_nc():
    global _NC_CACHE
    if _NC_CACHE is None:
        _NC_CACHE = _build()
    return _NC_CACHE


def _in_maps(x, W_exp, b_exp, W_con, b_con):
    wpack = np.concatenate(
        [W_exp[:H, :].T, (0.1 * W_exp[H:, :]).T, W_con.T], axis=1
    ).astype(np.float32)
    wpack = np.ascontiguousarray(wpack)
    bpack = np.stack(
        [-0.9 * b_exp[H:], b_exp[H:], b_exp[:H], b_con], axis=1
    ).astype(np.float32)
    bpack = np.ascontiguousarray(bpack)

    maps = []
    for c in range(NCORES):
        bb, nh = c // 2, c % 2
        xs = x[bb, :, nh * NLOC : (nh + 1) * NLOC, :]  # [T, NLOC, H]
        xT = np.ascontiguousarray(xs.transpose(2, 1, 0)).reshape(H, NBLK, TOK)
        maps.append(
            {
                "xt": xT,
                "wpack": wpack,
                "bpack": bpack,
            }
        )
    return maps


def run_spmd(x, W_exp, b_exp, W_con, b_con, **spmd_kwargs):
    """Run the 8-core kernel; returns (full_output, BassKernelResults)."""
    maps = _in_maps(x, W_exp, b_exp, W_con, b_con)
    res = run_bass_kernel_spmd(
        _get_nc(), maps, core_ids=list(range(NCORES)), **spmd_kwargs
    )
    out = np.empty((B, T, N, H), dtype=np.float32)
    for c in range(NCORES):
        bb, nh = c // 2, c % 2
        oT = res.results[c]["out"].reshape(H, NLOC, T)
        out[bb, :, nh * NLOC : (nh + 1) * NLOC, :] = oT.transpose(2, 1, 0)
    return out, res


def kernel(spatial_temporal_representation, W_exp, b_exp, W_con, b_con):
    out, _ = run_spmd(
        np.asarray(spatial_temporal_representation, dtype=np.float32),
        np.asarray(W_exp, dtype=np.float32),
        np.asarray(b_exp, dtype=np.float32),
        np.asarray(W_con, dtype=np.float32),
        np.asarray(b_con, dtype=np.float32),
    )
    return out



# revision 2
# speedup vs baseline: 1.2606x; 1.2606x over previous
"""TRN2 Bass kernel v2 for the ConceptualMambaBlock problem.

Math (reference):
    x: [B=4, T=96, N=512, H=128] f32
    expanded = x @ W_exp.T + b_exp            # [B,T,N,2H]
    primary, gating = split(expanded, 2, -1)
    s_t = 0.9*s_{t-1} + 0.1*gating_t          # EMA along T
    out = (primary * sigmoid(s)) @ W_con.T + b_con

v2 strategy (measured-HW-rate driven):
  - All matmul operands bf16 (fp32r was ~2.6x slower per column on HW);
    DMA I/O bf16 both ways (host converts); PSUM accum stays f32.
  - R=1536-column blocks (16 nodes x 96 t) so scan/stt/sigmoid run as
    single big instructions (amortize per-instr overheads).
  - PSUM (8 banks): two 3-bank regions, role-shared per block
    (mm1g -> scan -> mm1p -> stt/p-evac), + 2 one-bank po tiles for mm2.
  - DVE scan is the wall (2.07ns/col, no fast mode): keep DVE otherwise
    light by splitting the gate-mul: DVE-path blocks use a fused
    stt ((pp+b1p)*sig, 1.04ns/col); GPS-path blocks evacuate primary via
    ACT identity(+b1p)->bf16 and multiply on GPSIMD (TT bf16).
  - Gating bias via sigma-shift: scan unbiased gating; add -0.9*b_g to
    t=0 columns (ACT fixup on strided PSUM view); +b_g folded into the
    sigmoid bias port. 0.1 folded into W_exp gating half on host.
  - Output bf16 [H, N_local, T] per core; host upcasts + transposes.
"""

import numpy as np
import ml_dtypes

import concourse.bacc as bacc
import concourse.bass as bass  # noqa: F401
import concourse.mybir as mybir
import concourse.tile as tile
from concourse.bass_utils import run_bass_kernel_spmd

F32 = mybir.dt.float32
BF16 = mybir.dt.bfloat16
AF = mybir.ActivationFunctionType
ALU = mybir.AluOpType

B, T, N, H = 4, 96, 512, 128
NCORES = 8
NLOC = N // 2            # 256 nodes per core
R = 1536                 # columns per block (16 nodes x 96 t)
NBLK = NLOC * T // R     # 16 blocks per core
SEG = R // T             # 16 segments (nodes) per block
CH = 512                 # matmul chunk (one PSUM bank)
NCH = R // CH            # 3 chunks per block
DG = 2                   # blocks per DMA group
NDG = NBLK // DG         # 8 DMA groups

# Gate-mul path per block: True -> ACT p-evac + GPSIMD multiply,
# False -> fused stt on DVE.  ~7/16 on the GPS path balances
# DVE(scan+stt) against ACT(sig+evac+fix+p-evac) and GPSIMD.
GPS_PATH = [i % 16 in (1, 3, 5, 7, 9, 11, 13) for i in range(16)]

_NC_CACHE = None


def _build():
    nc = bacc.Bacc()

    xt_h = nc.dram_tensor("xt", [H, NBLK, R], BF16, kind="ExternalInput")
    wpack_h = nc.dram_tensor("wpack", [H, 3 * H], BF16, kind="ExternalInput")
    bpack_h = nc.dram_tensor("bpack", [H, 4], F32, kind="ExternalInput")
    out_h = nc.dram_tensor("out", [H, NBLK, R], BF16, kind="ExternalOutput")

    with tile.TileContext(nc) as tc:
        with (
            tc.tile_pool(name="consts", bufs=1) as cp,
            tc.tile_pool(name="io", bufs=1) as io,
            tc.tile_pool(name="mid", bufs=1) as mid,
            tc.tile_pool(name="ps", bufs=1, space="PSUM") as ps,
        ):
            wpack_sb = cp.tile([H, 3 * H], BF16, tag="wpack")
            nc.sync.dma_start(out=wpack_sb[:], in_=wpack_h[:, :])
            bpack_sb = cp.tile([H, 4], F32, tag="bpack")
            nc.sync.dma_start(out=bpack_sb[:], in_=bpack_h[:, :])
            w1g_sb = wpack_sb[:, 0:H]
            w1p_sb = wpack_sb[:, H:2 * H]
            w2_sb = wpack_sb[:, 2 * H:3 * H]
            bg_sb = bpack_sb[:, 0:1]     # +b_g (sigmoid bias port)
            b1p_sb = bpack_sb[:, 1:2]    # primary bias
            b2_sb = bpack_sb[:, 2:3]     # output bias (evac bias)
            bneg_sb = bpack_sb[:, 3:4]   # -0.9*b_g (t=0 fixup)

            mask_sb = cp.tile([H, SEG, T], F32, tag="mask")
            nc.gpsimd.memset(mask_sb[:], 0.9)
            nc.gpsimd.memset(mask_sb[:, :, 0:1], 0.0)
            mask2d = mask_sb[:].rearrange("p a b -> p (a b)")

            # Two 3-bank PSUM regions (role-shared: gating then primary).
            regions = [
                ps.tile([H, R], F32, tag=f"reg{i}", name=f"reg{i}")
                for i in range(2)
            ]
            state = {}

            def mm1g_fix_scan(k):
                reg = regions[k % 2]
                xk = state[k]["x"]
                for j in range(NCH):
                    nc.tensor.matmul(
                        reg[:, j * CH:(j + 1) * CH], lhsT=w1g_sb,
                        rhs=xk[:, j * CH:(j + 1) * CH], start=True, stop=True,
                    )
                # sigma-shift fixup: -0.9*b_g onto the t=0 columns
                pgc = reg[:].rearrange("p (a b) -> p a b", b=T)[:, :, 0:1]
                nc.scalar.activation(pgc, pgc, AF.Identity, bias=bneg_sb, scale=1.0)
                s = mid.tile([H, R], F32, tag="s", bufs=2, name="s")
                nc.vector.tensor_tensor_scan(
                    out=s[:], data0=mask2d, data1=reg[:],
                    initial=0.0, op0=ALU.mult, op1=ALU.add,
                )
                state[k]["s"] = s

            def sig_mm1p_mul(k):
                # sigmoid(k) on ACT
                sg = mid.tile([H, R], BF16, tag="sg", bufs=2, name="sg")
                nc.scalar.activation(
                    sg[:], state[k]["s"][:], AF.Sigmoid, bias=bg_sb, scale=1.0
                )
                # primary matmul into the same region (scan(k) already read it)
                reg = regions[k % 2]
                xk = state[k]["x"]
                for j in range(NCH):
                    nc.tensor.matmul(
                        reg[:, j * CH:(j + 1) * CH], lhsT=w1p_sb,
                        rhs=xk[:, j * CH:(j + 1) * CH], start=True, stop=True,
                    )
                y = mid.tile([H, R], BF16, tag="y", bufs=2, name="y")
                if GPS_PATH[k]:
                    ph = mid.tile([H, R], BF16, tag="ph", bufs=2, name="ph")
                    nc.scalar.activation(
                        ph[:], reg[:], AF.Identity, bias=b1p_sb, scale=1.0
                    )
                    nc.gpsimd.tensor_tensor(
                        out=y[:], in0=ph[:], in1=sg[:], op=ALU.mult
                    )
                else:
                    nc.vector.scalar_tensor_tensor(
                        out=y[:], in0=reg[:], scalar=b1p_sb, in1=sg[:],
                        op0=ALU.add, op1=ALU.mult,
                    )
                state[k]["y"] = y

            def mm2_evac(k):
                y, ob = state[k]["y"], state[k]["ob"]
                base = (k % DG) * R
                for j in range(NCH):
                    po = ps.tile([H, CH], F32, tag="po", name="po", bufs=2)
                    nc.tensor.matmul(
                        po[:], lhsT=w2_sb, rhs=y[:, j * CH:(j + 1) * CH],
                        start=True, stop=True,
                    )
                    nc.scalar.activation(
                        ob[:, base + j * CH:base + (j + 1) * CH],
                        po[:], AF.Identity, bias=b2_sb, scale=1.0,
                    )
                if k % DG == DG - 1:
                    gi = k // DG
                    nc.gpsimd.dma_start(
                        out=out_h[:, gi * DG:(gi + 1) * DG, :],
                        in_=state[k]["obt"][:],
                    )
                del state[k]

            # --- software pipeline (1-iter lead on loads) ---
            xgs = {}
            obt = None
            for k in range(NBLK + 2):
                # prefetch the x group needed at iter k+1
                gneed = (k + 1) // DG
                if gneed < NDG and gneed not in xgs:
                    xg = io.tile([H, DG, R], BF16, tag="xg", bufs=3, name="xg")
                    nc.sync.dma_start(
                        out=xg[:], in_=xt_h[:, gneed * DG:(gneed + 1) * DG, :]
                    )
                    xgs[gneed] = xg
                if k < NBLK:
                    gi = k // DG
                    if k % DG == 0:
                        obt = io.tile([H, DG * R], BF16, tag="ob", bufs=2,
                                      name="ob")
                    xg = xgs[gi]
                    state[k] = {
                        "x": xg[:].rearrange("p a b -> p (a b)")[
                            :, (k % DG) * R:(k % DG + 1) * R],
                        "ob": obt[:],
                        "obt": obt,
                    }
                    mm1g_fix_scan(k)
                if 0 <= k - 1 < NBLK:
                    sig_mm1p_mul(k - 1)
                if 0 <= k - 2 < NBLK:
                    mm2_evac(k - 2)

    nc.finalize()
    return nc


def _get_nc():
    global _NC_CACHE
    if _NC_CACHE is None:
        _NC_CACHE = _build()
    return _NC_CACHE


def _in_maps(x, W_exp, b_exp, W_con, b_con):
    bf = ml_dtypes.bfloat16
    wpack = np.concatenate(
        [(0.1 * W_exp[H:, :]).T, W_exp[:H, :].T, W_con.T], axis=1
    ).astype(bf)
    wpack = np.ascontiguousarray(wpack)
    bpack = np.stack(
        [b_exp[H:], b_exp[:H], b_con, -0.9 * b_exp[H:]], axis=1
    ).astype(np.float32)
    bpack = np.ascontiguousarray(bpack)

    maps = []
    for c in range(NCORES):
        bb, nh = c // 2, c % 2
        xs = x[bb, :, nh * NLOC:(nh + 1) * NLOC, :]  # [T, NLOC, H]
        xT = np.ascontiguousarray(
            xs.transpose(2, 1, 0).astype(bf)
        ).reshape(H, NBLK, R)
        maps.append({"xt": xT, "wpack": wpack, "bpack": bpack})
    return maps


def run_spmd(x, W_exp, b_exp, W_con, b_con, **spmd_kwargs):
    """Run the 8-core kernel; returns (full_output, BassKernelResults)."""
    maps = _in_maps(x, W_exp, b_exp, W_con, b_con)
    res = run_bass_kernel_spmd(
        _get_nc(), maps, core_ids=list(range(NCORES)), **spmd_kwargs
    )
    out = np.empty((B, T, N, H), dtype=np.float32)
    for c in range(NCORES):
        bb, nh = c // 2, c % 2
        oT = res.results[c]["out"].reshape(H, NLOC, T).astype(np.float32)
        out[bb, :, nh * NLOC:(nh + 1) * NLOC, :] = oT.transpose(2, 1, 0)
    return out, res


def kernel(spatial_temporal_representation, W_exp, b_exp, W_con, b_con):
    out, _ = run_spmd(
        np.asarray(spatial_temporal_representation, dtype=np.float32),
        np.asarray(W_exp, dtype=np.float32),
        np.asarray(b_exp, dtype=np.float32),
        np.asarray(W_con, dtype=np.float32),
        np.asarray(b_con, dtype=np.float32),
    )
    return out


# revision 3
# speedup vs baseline: 1.2656x; 1.0040x over previous
"""TRN2 Bass kernel v2 for the ConceptualMambaBlock problem.

Math (reference):
    x: [B=4, T=96, N=512, H=128] f32
    expanded = x @ W_exp.T + b_exp            # [B,T,N,2H]
    primary, gating = split(expanded, 2, -1)
    s_t = 0.9*s_{t-1} + 0.1*gating_t          # EMA along T
    out = (primary * sigmoid(s)) @ W_con.T + b_con

v2 strategy (measured-HW-rate driven):
  - All matmul operands bf16 (fp32r was ~2.6x slower per column on HW);
    DMA I/O bf16 both ways (host converts); PSUM accum stays f32.
  - R=1536-column blocks (16 nodes x 96 t) so scan/stt/sigmoid run as
    single big instructions (amortize per-instr overheads).
  - PSUM (8 banks): two 3-bank regions, role-shared per block
    (mm1g -> scan -> mm1p -> stt/p-evac), + 2 one-bank po tiles for mm2.
  - DVE scan is the wall (2.07ns/col, no fast mode): keep DVE otherwise
    light by splitting the gate-mul: DVE-path blocks use a fused
    stt ((pp+b1p)*sig, 1.04ns/col); GPS-path blocks evacuate primary via
    ACT identity(+b1p)->bf16 and multiply on GPSIMD (TT bf16).
  - Gating bias via sigma-shift: scan unbiased gating; add -0.9*b_g to
    t=0 columns (ACT fixup on strided PSUM view); +b_g folded into the
    sigmoid bias port. 0.1 folded into W_exp gating half on host.
  - Output bf16 [H, N_local, T] per core; host upcasts + transposes.
"""

import numpy as np
import ml_dtypes

import concourse.bacc as bacc
import concourse.bass as bass  # noqa: F401
import concourse.mybir as mybir
import concourse.tile as tile
from concourse.bass_utils import run_bass_kernel_spmd

F32 = mybir.dt.float32
BF16 = mybir.dt.bfloat16
AF = mybir.ActivationFunctionType
ALU = mybir.AluOpType

B, T, N, H = 4, 96, 512, 128
NCORES = 8
NLOC = N // 2            # 256 nodes per core
R = 1536                 # columns per block (16 nodes x 96 t)
NBLK = NLOC * T // R     # 16 blocks per core
SEG = R // T             # 16 segments (nodes) per block
CH = 512                 # matmul chunk (one PSUM bank)
NCH = R // CH            # 3 chunks per block
DG = 2                   # blocks per DMA group
NDG = NBLK // DG         # 8 DMA groups

# Gate-mul path per block: True -> ACT p-evac + GPSIMD multiply,
# False -> fused stt on DVE.  ~7/16 on the GPS path balances
# DVE(scan+stt) against ACT(sig+evac+fix+p-evac) and GPSIMD.
GPS_PATH = [i % 16 in (1, 3, 5, 7, 9, 11, 13) for i in range(16)]

_NC_CACHE = None


def _build():
    nc = bacc.Bacc()

    xt_h = nc.dram_tensor("xt", [H, NBLK, R], BF16, kind="ExternalInput")
    wpack_h = nc.dram_tensor("wpack", [H, 3 * H], BF16, kind="ExternalInput")
    bpack_h = nc.dram_tensor("bpack", [H, 4], F32, kind="ExternalInput")
    bneg_h = nc.dram_tensor("bneg", [1, H], BF16, kind="ExternalInput")
    out_h = nc.dram_tensor("out", [H, NBLK, R], BF16, kind="ExternalOutput")

    with tile.TileContext(nc) as tc:
        with (
            tc.tile_pool(name="consts", bufs=1) as cp,
            tc.tile_pool(name="io", bufs=1) as io,
            tc.tile_pool(name="mid", bufs=1) as mid,
            tc.tile_pool(name="ps", bufs=1, space="PSUM") as ps,
        ):
            wpack_sb = cp.tile([H, 3 * H], BF16, tag="wpack")
            nc.sync.dma_start(out=wpack_sb[:], in_=wpack_h[:, :])
            bpack_sb = cp.tile([H, 4], F32, tag="bpack")
            nc.sync.dma_start(out=bpack_sb[:], in_=bpack_h[:, :])
            bneg_sb = cp.tile([1, H], BF16, tag="bneg")
            nc.sync.dma_start(out=bneg_sb[:], in_=bneg_h[:, :])
            ones_sb = cp.tile([1, SEG], BF16, tag="ones")
            nc.gpsimd.memset(ones_sb[:], 1.0)
            w1g_sb = wpack_sb[:, 0:H]
            w1p_sb = wpack_sb[:, H:2 * H]
            w2_sb = wpack_sb[:, 2 * H:3 * H]
            bg_sb = bpack_sb[:, 0:1]     # +b_g (sigmoid bias port)
            b1p_sb = bpack_sb[:, 1:2]    # primary bias
            b2_sb = bpack_sb[:, 2:3]     # output bias (evac bias)
            bnegf_sb = bpack_sb[:, 3:4]  # -0.9*b_g (unused: PE fixmm)

            mask_sb = cp.tile([H, SEG, T], F32, tag="mask")
            nc.gpsimd.memset(mask_sb[:], 0.9)
            nc.gpsimd.memset(mask_sb[:, :, 0:1], 0.0)
            mask2d = mask_sb[:].rearrange("p a b -> p (a b)")

            # Two 3-bank PSUM regions (role-shared: gating then primary).
            regions = [
                ps.tile([H, R], F32, tag=f"reg{i}", name=f"reg{i}")
                for i in range(2)
            ]
            state = {}

            def mm1g_fix_scan(k):
                reg = regions[k % 2]
                xk = state[k]["x"]
                for j in range(NCH):
                    nc.tensor.matmul(
                        reg[:, j * CH:(j + 1) * CH], lhsT=w1g_sb,
                        rhs=xk[:, j * CH:(j + 1) * CH], start=True, stop=True,
                    )
                # sigma-shift fixup: -0.9*b_g onto the t=0 columns via a
                # rank-1 matmul so the scan's dep chain stays PE-internal
                pgc = reg[:].rearrange("p (a b) -> p a b", b=T)[:, :, 0:1]
                nc.tensor.matmul(
                    pgc, lhsT=bneg_sb[:], rhs=ones_sb[:],
                    start=False, stop=True, skip_group_check=True,
                )
                s = mid.tile([H, R], F32, tag="s", bufs=3, name="s")
                nc.vector.tensor_tensor_scan(
                    out=s[:], data0=mask2d, data1=reg[:],
                    initial=0.0, op0=ALU.mult, op1=ALU.add,
                )
                state[k]["s"] = s

            def sig_mm1p_mul(k):
                # sigmoid(k) on ACT
                sg = mid.tile([H, R], BF16, tag="sg", bufs=4, name="sg")
                nc.scalar.activation(
                    sg[:], state[k]["s"][:], AF.Sigmoid, bias=bg_sb, scale=1.0
                )
                # primary matmul into the same region (scan(k) already read it)
                reg = regions[k % 2]
                xk = state[k]["x"]
                for j in range(NCH):
                    nc.tensor.matmul(
                        reg[:, j * CH:(j + 1) * CH], lhsT=w1p_sb,
                        rhs=xk[:, j * CH:(j + 1) * CH], start=True, stop=True,
                    )
                y = mid.tile([H, R], BF16, tag="y", bufs=3, name="y")
                if GPS_PATH[k]:
                    ph = mid.tile([H, R], BF16, tag="ph", bufs=3, name="ph")
                    nc.scalar.activation(
                        ph[:], reg[:], AF.Identity, bias=b1p_sb, scale=1.0
                    )
                    nc.gpsimd.tensor_tensor(
                        out=y[:], in0=ph[:], in1=sg[:], op=ALU.mult
                    )
                else:
                    nc.vector.scalar_tensor_tensor(
                        out=y[:], in0=reg[:], scalar=b1p_sb, in1=sg[:],
                        op0=ALU.add, op1=ALU.mult,
                    )
                state[k]["y"] = y

            def mm2_evac(k):
                y, ob = state[k]["y"], state[k]["ob"]
                base = (k % DG) * R
                for j in range(NCH):
                    po = ps.tile([H, CH], F32, tag="po", name="po", bufs=2)
                    nc.tensor.matmul(
                        po[:], lhsT=w2_sb, rhs=y[:, j * CH:(j + 1) * CH],
                        start=True, stop=True,
                    )
                    dst = ob[:, base + j * CH:base + (j + 1) * CH]
                    if j == 0:
                        nc.vector.tensor_scalar_add(dst, po[:], b2_sb)
                    else:
                        nc.scalar.activation(
                            dst, po[:], AF.Identity, bias=b2_sb, scale=1.0,
                        )
                if k % DG == DG - 1:
                    gi = k // DG
                    nc.gpsimd.dma_start(
                        out=out_h[:, gi * DG:(gi + 1) * DG, :],
                        in_=state[k]["obt"][:],
                    )
                del state[k]

            # --- software pipeline (1-iter lead on loads) ---
            xgs = {}
            obt = None
            for k in range(NBLK + 2):
                # prefetch the x group needed at iter k+1
                gneed = (k + 1) // DG
                if gneed < NDG and gneed not in xgs:
                    xg = io.tile([H, DG, R], BF16, tag="xg", bufs=3, name="xg")
                    nc.sync.dma_start(
                        out=xg[:], in_=xt_h[:, gneed * DG:(gneed + 1) * DG, :]
                    )
                    xgs[gneed] = xg
                if k < NBLK:
                    gi = k // DG
                    if k % DG == 0:
                        obt = io.tile([H, DG * R], BF16, tag="ob", bufs=2,
                                      name="ob")
                    xg = xgs[gi]
                    state[k] = {
                        "x": xg[:].rearrange("p a b -> p (a b)")[
                            :, (k % DG) * R:(k % DG + 1) * R],
                        "ob": obt[:],
                        "obt": obt,
                    }
                    mm1g_fix_scan(k)
                if 0 <= k - 1 < NBLK:
                    sig_mm1p_mul(k - 1)
                if 0 <= k - 2 < NBLK:
                    mm2_evac(k - 2)

    nc.finalize()
    return nc


def _get_nc():
    global _NC_CACHE
    if _NC_CACHE is None:
        _NC_CACHE = _build()
    return _NC_CACHE


def _in_maps(x, W_exp, b_exp, W_con, b_con):
    bf = ml_dtypes.bfloat16
    wpack = np.concatenate(
        [(0.1 * W_exp[H:, :]).T, W_exp[:H, :].T, W_con.T], axis=1
    ).astype(bf)
    wpack = np.ascontiguousarray(wpack)
    bpack = np.stack(
        [b_exp[H:], b_exp[:H], b_con, -0.9 * b_exp[H:]], axis=1
    ).astype(np.float32)
    bpack = np.ascontiguousarray(bpack)
    bneg = np.ascontiguousarray((-0.9 * b_exp[H:]).astype(bf).reshape(1, H))

    maps = []
    for c in range(NCORES):
        bb, nh = c // 2, c % 2
        xs = x[bb, :, nh * NLOC:(nh + 1) * NLOC, :]  # [T, NLOC, H]
        xT = np.ascontiguousarray(
            xs.transpose(2, 1, 0).astype(bf)
        ).reshape(H, NBLK, R)
        maps.append({"xt": xT, "wpack": wpack, "bpack": bpack, "bneg": bneg})
    return maps


def run_spmd(x, W_exp, b_exp, W_con, b_con, **spmd_kwargs):
    """Run the 8-core kernel; returns (full_output, BassKernelResults)."""
    maps = _in_maps(x, W_exp, b_exp, W_con, b_con)
    res = run_bass_kernel_spmd(
        _get_nc(), maps, core_ids=list(range(NCORES)), **spmd_kwargs
    )
    out = np.empty((B, T, N, H), dtype=np.float32)
    for c in range(NCORES):
        bb, nh = c // 2, c % 2
        oT = res.results[c]["out"].reshape(H, NLOC, T).astype(np.float32)
        out[bb, :, nh * NLOC:(nh + 1) * NLOC, :] = oT.transpose(2, 1, 0)
    return out, res


def kernel(spatial_temporal_representation, W_exp, b_exp, W_con, b_con):
    out, _ = run_spmd(
        np.asarray(spatial_temporal_representation, dtype=np.float32),
        np.asarray(W_exp, dtype=np.float32),
        np.asarray(b_exp, dtype=np.float32),
        np.asarray(W_con, dtype=np.float32),
        np.asarray(b_con, dtype=np.float32),
    )
    return out


# revision 4
# speedup vs baseline: 1.2999x; 1.0271x over previous
"""TRN2 Bass kernel v2 for the ConceptualMambaBlock problem.

Math (reference):
    x: [B=4, T=96, N=512, H=128] f32
    expanded = x @ W_exp.T + b_exp            # [B,T,N,2H]
    primary, gating = split(expanded, 2, -1)
    s_t = 0.9*s_{t-1} + 0.1*gating_t          # EMA along T
    out = (primary * sigmoid(s)) @ W_con.T + b_con

v2 strategy (measured-HW-rate driven):
  - All matmul operands bf16 (fp32r was ~2.6x slower per column on HW);
    DMA I/O bf16 both ways (host converts); PSUM accum stays f32.
  - R=1536-column blocks (16 nodes x 96 t) so scan/stt/sigmoid run as
    single big instructions (amortize per-instr overheads).
  - PSUM (8 banks): two 3-bank regions, role-shared per block
    (mm1g -> scan -> mm1p -> stt/p-evac), + 2 one-bank po tiles for mm2.
  - DVE scan is the wall (2.07ns/col, no fast mode): keep DVE otherwise
    light by splitting the gate-mul: DVE-path blocks use a fused
    stt ((pp+b1p)*sig, 1.04ns/col); GPS-path blocks evacuate primary via
    ACT identity(+b1p)->bf16 and multiply on GPSIMD (TT bf16).
  - Gating bias via sigma-shift: scan unbiased gating; add -0.9*b_g to
    t=0 columns (ACT fixup on strided PSUM view); +b_g folded into the
    sigmoid bias port. 0.1 folded into W_exp gating half on host.
  - Output bf16 [H, N_local, T] per core; host upcasts + transposes.
"""

import numpy as np
import ml_dtypes

import concourse.bacc as bacc
import concourse.bass as bass  # noqa: F401
import concourse.mybir as mybir
import concourse.tile as tile
from concourse.bass_utils import run_bass_kernel_spmd

F32 = mybir.dt.float32
BF16 = mybir.dt.bfloat16
AF = mybir.ActivationFunctionType
ALU = mybir.AluOpType

B, T, N, H = 4, 96, 512, 128
NCORES = 8
NLOC = N // 2            # 256 nodes per core
R = 1536                 # columns per block (16 nodes x 96 t)
NBLK = NLOC * T // R     # 16 blocks per core
SEG = R // T             # 16 segments (nodes) per block
CH = 512                 # matmul chunk (one PSUM bank)
NCH = R // CH            # 3 chunks per block
DG = 2                   # blocks per DMA group
NDG = NBLK // DG         # 8 DMA groups

# Gate-mul path per block: True -> ACT p-evac + GPSIMD multiply,
# False -> fused stt on DVE.  ~7/16 on the GPS path balances
# DVE(scan+stt) against ACT(sig+evac+fix+p-evac) and GPSIMD.
GPS_PATH = [i % 16 in (1, 3, 5, 7, 9, 11, 13) for i in range(16)]

_NC_CACHE = None


def _build():
    nc = bacc.Bacc()

    xt_h = nc.dram_tensor("xt", [H, NBLK, R], BF16, kind="ExternalInput")
    wpack_h = nc.dram_tensor("wpack", [H, 3 * H], BF16, kind="ExternalInput")
    bpack_h = nc.dram_tensor("bpack", [H, 4], F32, kind="ExternalInput")
    bneg_h = nc.dram_tensor("bneg", [1, H], BF16, kind="ExternalInput")
    out_h = nc.dram_tensor("out", [H, NBLK, R], BF16, kind="ExternalOutput")

    with tile.TileContext(nc) as tc:
        with (
            tc.tile_pool(name="consts", bufs=1) as cp,
            tc.tile_pool(name="io", bufs=1) as io,
            tc.tile_pool(name="mid", bufs=1) as mid,
            tc.tile_pool(name="ps", bufs=1, space="PSUM") as ps,
        ):
            wpack_sb = cp.tile([H, 3 * H], BF16, tag="wpack")
            nc.sync.dma_start(out=wpack_sb[:], in_=wpack_h[:, :])
            bpack_sb = cp.tile([H, 4], F32, tag="bpack")
            nc.sync.dma_start(out=bpack_sb[:], in_=bpack_h[:, :])
            bneg_sb = cp.tile([1, H], BF16, tag="bneg")
            nc.sync.dma_start(out=bneg_sb[:], in_=bneg_h[:, :])
            ones_sb = cp.tile([1, SEG], BF16, tag="ones")
            nc.gpsimd.memset(ones_sb[:], 1.0)
            w1g_sb = wpack_sb[:, 0:H]
            w1p_sb = wpack_sb[:, H:2 * H]
            w2_sb = wpack_sb[:, 2 * H:3 * H]
            bg_sb = bpack_sb[:, 0:1]     # +b_g (sigmoid bias port)
            b1p_sb = bpack_sb[:, 1:2]    # primary bias
            b2_sb = bpack_sb[:, 2:3]     # output bias (evac bias)
            bnegf_sb = bpack_sb[:, 3:4]  # -0.9*b_g (unused: PE fixmm)

            mask_sb = cp.tile([H, SEG, T], F32, tag="mask")
            nc.gpsimd.memset(mask_sb[:], 0.9)
            nc.gpsimd.memset(mask_sb[:, :, 0:1], 0.0)
            mask2d = mask_sb[:].rearrange("p a b -> p (a b)")

            # Two 3-bank PSUM regions (role-shared: gating then primary).
            regions = [
                ps.tile([H, R], F32, tag=f"reg{i}", name=f"reg{i}")
                for i in range(2)
            ]
            state = {}

            def mm1g_fix_scan(k):
                reg = regions[k % 2]
                xk = state[k]["x"]
                for j in range(NCH):
                    nc.tensor.matmul(
                        reg[:, j * CH:(j + 1) * CH], lhsT=w1g_sb,
                        rhs=xk[:, j * CH:(j + 1) * CH], start=True, stop=True,
                    )
                # sigma-shift fixup: -0.9*b_g onto the t=0 columns via a
                # rank-1 matmul so the scan's dep chain stays PE-internal
                pgc = reg[:].rearrange("p (a b) -> p a b", b=T)[:, :, 0:1]
                nc.tensor.matmul(
                    pgc, lhsT=bneg_sb[:], rhs=ones_sb[:],
                    start=False, stop=True, skip_group_check=True,
                )
                s = mid.tile([H, R], F32, tag="s", bufs=3, name="s")
                nc.vector.tensor_tensor_scan(
                    out=s[:], data0=mask2d, data1=reg[:],
                    initial=0.0, op0=ALU.mult, op1=ALU.add,
                )
                state[k]["s"] = s

            def sig_mm1p_mul(k):
                # sigmoid(k) on ACT
                sg = mid.tile([H, R], BF16, tag="sg", bufs=4, name="sg")
                nc.scalar.activation(
                    sg[:], state[k]["s"][:], AF.Sigmoid, bias=bg_sb, scale=1.0
                )
                # primary matmul into the same region (scan(k) already read it)
                reg = regions[k % 2]
                xk = state[k]["x"]
                for j in range(NCH):
                    nc.tensor.matmul(
                        reg[:, j * CH:(j + 1) * CH], lhsT=w1p_sb,
                        rhs=xk[:, j * CH:(j + 1) * CH], start=True, stop=True,
                    )
                y = mid.tile([H, R], BF16, tag="y", bufs=3, name="y")
                if GPS_PATH[k]:
                    ph = mid.tile([H, R], BF16, tag="ph", bufs=3, name="ph")
                    nc.scalar.activation(
                        ph[:], reg[:], AF.Identity, bias=b1p_sb, scale=1.0
                    )
                    nc.gpsimd.tensor_tensor(
                        out=y[:], in0=ph[:], in1=sg[:], op=ALU.mult
                    )
                else:
                    nc.vector.scalar_tensor_tensor(
                        out=y[:], in0=reg[:], scalar=b1p_sb, in1=sg[:],
                        op0=ALU.add, op1=ALU.mult,
                    )
                state[k]["y"] = y

            def mm2_evac(k):
                y, ob = state[k]["y"], state[k]["ob"]
                base = (k % DG) * R
                for j in range(NCH):
                    po = ps.tile([H, CH], F32, tag="po", name="po", bufs=2)
                    nc.tensor.matmul(
                        po[:], lhsT=w2_sb, rhs=y[:, j * CH:(j + 1) * CH],
                        start=True, stop=True,
                    )
                    dst = ob[:, base + j * CH:base + (j + 1) * CH]
                    if j == 0 and GPS_PATH[k]:
                        # fill the DVE hole on GPS-path iterations
                        nc.vector.tensor_scalar_add(dst, po[:], b2_sb)
                    else:
                        nc.scalar.activation(
                            dst, po[:], AF.Identity, bias=b2_sb, scale=1.0,
                        )
                if k % DG == DG - 1:
                    gi = k // DG
                    nc.gpsimd.dma_start(
                        out=out_h[:, gi * DG:(gi + 1) * DG, :],
                        in_=state[k]["obt"][:],
                    )
                del state[k]

            # --- software pipeline (1-iter lead on loads) ---
            xgs = {}
            obt = None
            for k in range(NBLK + 2):
                # prefetch the x group needed at iter k+1
                gneed = (k + 1) // DG
                if gneed < NDG and gneed not in xgs:
                    xg = io.tile([H, DG, R], BF16, tag="xg", bufs=3, name="xg")
                    nc.sync.dma_start(
                        out=xg[:], in_=xt_h[:, gneed * DG:(gneed + 1) * DG, :]
                    )
                    xgs[gneed] = xg
                if k < NBLK:
                    gi = k // DG
                    if k % DG == 0:
                        obt = io.tile([H, DG * R], BF16, tag="ob", bufs=2,
                                      name="ob")
                    xg = xgs[gi]
                    state[k] = {
                        "x": xg[:].rearrange("p a b -> p (a b)")[
                            :, (k % DG) * R:(k % DG + 1) * R],
                        "ob": obt[:],
                        "obt": obt,
                    }
                    mm1g_fix_scan(k)
                if 0 <= k - 1 < NBLK:
                    sig_mm1p_mul(k - 1)
                if 0 <= k - 2 < NBLK:
                    mm2_evac(k - 2)

    nc.finalize()
    return nc


def _get_nc():
    global _NC_CACHE
    if _NC_CACHE is None:
        _NC_CACHE = _build()
    return _NC_CACHE


def _in_maps(x, W_exp, b_exp, W_con, b_con):
    bf = ml_dtypes.bfloat16
    wpack = np.concatenate(
        [(0.1 * W_exp[H:, :]).T, W_exp[:H, :].T, W_con.T], axis=1
    ).astype(bf)
    wpack = np.ascontiguousarray(wpack)
    bpack = np.stack(
        [b_exp[H:], b_exp[:H], b_con, -0.9 * b_exp[H:]], axis=1
    ).astype(np.float32)
    bpack = np.ascontiguousarray(bpack)
    bneg = np.ascontiguousarray((-0.9 * b_exp[H:]).astype(bf).reshape(1, H))

    maps = []
    for c in range(NCORES):
        bb, nh = c // 2, c % 2
        xs = x[bb, :, nh * NLOC:(nh + 1) * NLOC, :]  # [T, NLOC, H]
        xT = np.ascontiguousarray(
            xs.transpose(2, 1, 0).astype(bf)
        ).reshape(H, NBLK, R)
        maps.append({"xt": xT, "wpack": wpack, "bpack": bpack, "bneg": bneg})
    return maps


def run_spmd(x, W_exp, b_exp, W_con, b_con, **spmd_kwargs):
    """Run the 8-core kernel; returns (full_output, BassKernelResults)."""
    maps = _in_maps(x, W_exp, b_exp, W_con, b_con)
    res = run_bass_kernel_spmd(
        _get_nc(), maps, core_ids=list(range(NCORES)), **spmd_kwargs
    )
    out = np.empty((B, T, N, H), dtype=np.float32)
    for c in range(NCORES):
        bb, nh = c // 2, c % 2
        oT = res.results[c]["out"].reshape(H, NLOC, T).astype(np.float32)
        out[bb, :, nh * NLOC:(nh + 1) * NLOC, :] = oT.transpose(2, 1, 0)
    return out, res


def kernel(spatial_temporal_representation, W_exp, b_exp, W_con, b_con):
    out, _ = run_spmd(
        np.asarray(spatial_temporal_representation, dtype=np.float32),
        np.asarray(W_exp, dtype=np.float32),
        np.asarray(b_exp, dtype=np.float32),
        np.asarray(W_con, dtype=np.float32),
        np.asarray(b_con, dtype=np.float32),
    )
    return out
